# revision 41
# baseline (speedup 1.0000x reference)
"""LSH cosine-of-Hamming retrieval kernel for 8 trn2 NeuronCores.

Math: reference computes cos((pi/d) * hamming(u, v)) for binary LSH codes
u = (emb1 @ r.T > 0), v = (emb2 @ r.T > 0), d = 1024 bits.
With +/-1 sign codes s_u = 2u-1, s_v = 2v-1:
    hamming = (d - s_u . s_v) / 2
    cos((pi/d) * hamming) = sin((pi/2d) * s_u.s_v)
u codes are stored as +/-1 fp8 (ACT Sign), v codes as +/-0.5 fp8 (DVE
is_gt/subtract), so psum P = 0.5 * s_u.s_v and out = sin((pi/d) * P).

Projection runs in ONE fp32r pass (the PE truncates operands to ~FP22
and products to similar precision; inputs are pre-rounded to 13
mantissa bits on the host so the input truncation is a no-op, leaving
~6e-5 sign-flip rate -> 9.6e-3 rel err, inside the 2e-2 budget). This
replaces the 3-pass bf16 hi/lo split projection: 64 instead of 192
projection matmuls (fp32r streams ~1.7 cycles/row vs bf16's 1.0, so
projection PE time drops 41us -> ~24us).

The binarize of the projection psum (4M fp32 values) is the side
constraint: GPSIMD cannot read PSUM and DVE 2x perf modes need 16-bit
or SBUF sources, so only DVE and ACT can drain codes, at 1 elem/cycle
with a ~0.3-0.5us fixed cost per instruction. Three consequences shape
the schedule:
- binarize in 2-bank groups (one instruction per two 128-bit chunks)
  to amortize the fixed cost (~16us/engine instead of ~22-26);
- code magnitude only needs to be uniform per (tensor, 512-wide
  j-block), because each main psum bank contracts exactly one u
  j-block against one v j-block — the per-half Sin scale absorbs the
  magnitude product. So u j0 is ACT Sign (+/-1) and everything else is
  DVE is_gt/sub (+/-0.5), keeping ACT free for the Sin stream;
- only u j0 + v j0 + v j1 are projected upfront; the remaining 20
  groups trail one-per-half through the main stream (DVE drains them
  with slack before first use, per the block order below).

Main phase: fp8 DoubleRow "vertical halves" (two adjacent 128-row
blocks x one 512-col quarter, one psum bank each), ordered by (u j,
v j) block so the needed-code set grows slowly; Sin on ACT with the
per-half scale, output DMAs alternating sync/gpsimd queues (tail on
sync so the SWDGE ring drains early). The first/last two halves use
per-bank Sin/DMA quanta to shorten recycle waits and the drain.

The PE clock (HAM) re-throttles to 1.2GHz after ~3.4us of accumulated
idle in its window, costing ~5-7us per event: 12 warm-up matmuls cover
the DMA-latency head, warm fillers plug the known arrival-wait sites
in the upfront phase, and input DMAs spread over three queues (r +
late e1 on sync, early e1 on scalar, e2 on gpsimd) keep per-queue
chains short so no early arrival lands late enough to stall the PE
past the window. Do NOT interleave main matmuls chunk-by-chunk into
the projection stream (chunk-pipelined opening): matmuls parked on
code semaphores while weight loads stream past trip the LDWEIGHTS
clobber below and intermittently corrupt whole output quadrants.

NOTE: do NOT dedupe back-to-back identical LDWEIGHTS here - on this
runtime the PE's reorder window executes pulled-ahead weight loads
while early matmuls are parked on unsatisfied code semaphores, and the
second matmul of a pair then runs with clobbered weights (reproducible
NaN quadrants in the first scheduled halves).

Sharding (2x4 grid over 8 cores): core k computes the [2048, 2048]
output block for emb1 rows [(k//4)*2048...] x emb2 rows [(k%4)*2048...];
r is replicated (collectives cost ~60us fixed here - not worth it).
Host prep is layout-only: transpose + fp22 pre-round.
"""

import sys

sys.path.insert(0, "/opt/trn_rl_repo")

import numpy as np

import concourse.bacc as bacc
import concourse.tile as tile
from concourse import mybir
from concourse.bass_utils import run_bass_kernel_spmd

N1, N2, D, B = 4096, 8192, 128, 1024  # emb1 rows, emb2 rows, dim, num_bits
G1, G2 = 2, 4
M1, M2 = N1 // G1, N2 // G2  # 2048 x 2048 output block per core
KC = B // 128  # 8 bit-chunks of 128
RW = 512  # projection row-chunk width
NW = 512  # main matmul psum tile width

_BUILD_CACHE = {}


def _build(scale: float):
    if scale in _BUILD_CACHE:
        return _BUILD_CACHE[scale]
    nc = bacc.Bacc("TRN2", target_bir_lowering=False, debug=False)
    f32 = mybir.dt.float32
    f32r = mybir.dt.float32r
    bf16 = mybir.dt.bfloat16
    fp8 = mybir.dt.float8e4
    A = mybir.AluOpType
    AF = mybir.ActivationFunctionType

    e1 = nc.declare_dram_parameter("e1", [D, M1], f32r, isOutput=False)
    e2 = nc.declare_dram_parameter("e2", [D, M2], f32r, isOutput=False)
    rr = nc.declare_dram_parameter("rr", [D, B], f32r, isOutput=False)
    out = nc.declare_dram_parameter("out", [M1, M2], bf16, isOutput=True)

    with tile.TileContext(nc) as tc:
        with (
            tc.tile_pool(name="const", bufs=1) as const_pool,
            tc.tile_pool(name="outs", bufs=6) as out_pool,
            tc.tile_pool(name="pproj", bufs=2, space="PSUM") as pp,
            tc.tile_pool(name="pmain", bufs=2, space="PSUM") as mp,
        ):
            r_sb = const_pool.tile([D, B], f32r)
            e1_sb = const_pool.tile([D, M1], f32r)
            e2_sb = const_pool.tile([D, M2], f32r)
            ut = const_pool.tile([128, KC, M1], fp8)
            vt = const_pool.tile([128, KC, M2], fp8)

            # Warm-up operand: read a corner of the (not-yet-written)
            # code tile — the psum result is discarded, so garbage is
            # fine, and skipping a memset lets the gpsimd queue issue
            # e2 j0 ~0.5us earlier and the warm-up start unconditionally.
            # The u j3 corner is binarized ~20us after the last filler
            # read, so the write-after-read order costs nothing.
            warm = ut

            # Input DMAs across three queues in parallel (issue-to-land
            # latency is ~4.8us fixed, transfers ~0.75us per 256KB piece):
            # sync carries r (the gate for every projection) + late e1,
            # scalar carries early e1, gpsimd (SWDGE: jittier first
            # landing, but idle until the output DMAs at ~21us) carries
            # all of e2. Short per-queue chains land the early arrivals
            # sooner and accumulate less jitter — a +3us-late early
            # arrival stalls the PE long enough to re-throttle the clock.
            nc.sync.dma_start(r_sb[:, 0:512], rr[:, 0:512])
            nc.scalar.dma_start(e1_sb[:, 0:RW], e1[:, 0:RW])
            nc.gpsimd.dma_start(e2_sb[:, 0:RW], e2[:, 0:RW])
            nc.sync.dma_start(r_sb[:, 512:], rr[:, 512:])
            nc.scalar.dma_start(e1_sb[:, RW : 2 * RW], e1[:, RW : 2 * RW])
            for j in (1, 2, 3):
                nc.gpsimd.dma_start(
                    e2_sb[:, j * RW : (j + 1) * RW], e2[:, j * RW : (j + 1) * RW]
                )
            for j in (2, 3):
                nc.sync.dma_start(
                    e1_sb[:, j * RW : (j + 1) * RW], e1[:, j * RW : (j + 1) * RW]
                )

            # HAM warm-up: the PE clock ramps to 2.4GHz only after ~3.5us
            # of sustained activity (trace: proj ran at 1.2GHz until
            # t~20us with only 8 warmups + a gap). Burn enough dummy
            # matmuls to (a) cover the >=3.5us busy window and (b) keep
            # the PE busy until the first input DMAs land (~12us), so
            # projection starts on a warm clock with no PE idle gap.
            for _ in range(12):
                wps = pp.tile([128, 2, RW], f32, name="pjtile", tag="pj")
                nc.tensor.matmul(
                    wps[:, 0, :],
                    warm[:, KC - 1, M1 - 128 : M1],
                    warm[:, KC - 1, M1 - RW : M1],
                    start=True,
                    stop=True,
                )

            # Engine assignment per (tensor, j-block): a code block's
            # magnitude only needs to be uniform within its own j-block,
            # because each main psum bank reads exactly one u j-block and
            # one v j-block — the per-half Sin scale absorbs the product
            # of magnitudes. ACT Sign gives +/-1, DVE is_gt/sub +/-0.5.
            # Only u j0 goes to ACT (needed first, before ACT has Sin
            # work); everything else goes to DVE — this keeps ACT free to
            # stream Sins with no binarize bursts delaying the main psum
            # recycle.
            def on_act(is_u, j):
                return is_u and j == 0

            def proj_group(is_u, j, a, in_mp=False):
                # Two 128-bit chunks (c=2a, 2a+1) into one 2-bank psum
                # tile, drained by a single binarize instruction: ACT has
                # ~494ns fixed cost per instruction, so 2-bank groups cut
                # the binarize engine time from ~22us to ~16us per engine
                # (the projection phase is binarize-bound, not PE-bound).
                # During the upfront phase the main pool is idle, so
                # alternating groups across both pools doubles the psum
                # recycle depth (binarize latency hides behind 4 tiles).
                src = e1_sb if is_u else e2_sb
                dst = ut if is_u else vt
                sl = slice(j * RW, (j + 1) * RW)
                if in_mp:
                    ps = mp.tile([128, 2, RW], f32, name="pmtile", tag="pm")
                else:
                    ps = pp.tile([128, 2, RW], f32, name="pjtile", tag="pj")
                for i in range(2):
                    cs = slice((2 * a + i) * 128, (2 * a + i + 1) * 128)
                    nc.tensor.matmul(
                        ps[:, i, :], r_sb[:, cs], src[:, sl], start=True, stop=True
                    )
                if on_act(is_u, j):
                    nc.scalar.activation(dst[:, 2 * a : 2 * a + 2, sl], ps[:], AF.Sign)
                else:
                    nc.vector.tensor_scalar(
                        dst[:, 2 * a : 2 * a + 2, sl], ps[:], 0.0, 0.5, A.is_gt, A.subtract
                    )

            def vert_half(a, pair, b, dmaq, fine=False):
                # "Vertical half": two adjacent 128-row blocks x one 512
                # column quarter, one psum bank per row block. The second
                # bank's matmuls only need the same (u j, v j) codes, so a
                # vert-half depends on exactly one u j-block and one v
                # j-block — main work can start as soon as u j0 + v j0
                # codes exist.
                ps = mp.tile([128, 2, NW], f32, name="pmtile", tag="pm")
                ot = out_pool.tile([128, 2 * NW], bf16)
                ns = slice(b * NW, (b + 1) * NW)
                # psum = u_mag * v_mag * (s_u . s_v); Sin(scale * psum)
                # needs scale = (pi/2d) / (u_mag * v_mag); ACT-binarized
                # blocks are +/-1, DVE ones +/-0.5.
                um = 1.0 if on_act(True, a) else 0.5
                vm = 1.0 if on_act(False, b) else 0.5
                sc = scale * 0.5 / (um * vm)
                for mi in range(2):
                    m = 4 * a + 2 * pair + mi
                    ms = slice(m * 128, (m + 1) * 128)
                    for s in range(KC // 2):
                        nc.tensor.matmul(
                            ps[:, mi, :],
                            ut[:, 2 * s : 2 * s + 2, ms],
                            vt[:, 2 * s : 2 * s + 2, ns],
                            start=(s == 0),
                            stop=(s == KC // 2 - 1),
                            perf_mode=mybir.MatmulPerfMode.DoubleRow,
                        )
                    if fine:
                        # last vert-halves: per-bank Sin/DMA so bank0's
                        # drain overlaps bank1's matmuls
                        nc.scalar.activation(
                            ot[:, mi * NW : (mi + 1) * NW], ps[:, mi, :], AF.Sin, scale=sc
                        )
                        dmaq.dma_start(out[ms, ns], ot[:, mi * NW : (mi + 1) * NW])
                if not fine:
                    nc.scalar.activation(ot[:], ps[:], AF.Sin, scale=sc)
                    for mi in range(2):
                        m = 4 * a + 2 * pair + mi
                        ms = slice(m * 128, (m + 1) * 128)
                        dmaq.dma_start(out[ms, ns], ot[:, mi * NW : (mi + 1) * NW])

            # Upfront projection: only what the first blocks need — u j0
            # (ACT), v j0 and v j1 (DVE), interleaved in DMA-arrival
            # order. The first vert-half waits only for u j0 + v j0.
            # None: warm filler matmuls at the known DMA-arrival stall
            # sites (e2 j0, r tail) — a 1-2.5us PE idle there can cross
            # the HAM MID window and re-throttle the clock to 1.2GHz for
            # ~7us; the fillers substitute busy-work for that idle.
            wfill = mp.tile([128, 2, RW], f32, name="pmtile", tag="pm")
            UP = [
                (True, 0, 0), (True, 0, 1), None, None, (False, 0, 0),
                (True, 0, 2), None, (False, 0, 1), (True, 0, 3), None,
                (False, 0, 2), (False, 0, 3), None, (False, 1, 0),
                (False, 1, 1), (False, 1, 2), (False, 1, 3),
            ]
            for g in UP:
                if g is None:
                    nc.tensor.matmul(
                        wfill[:, 0, :],
                        warm[:, KC - 1, M1 - 128 : M1],
                        warm[:, KC - 1, M1 - RW : M1],
                        start=True,
                        stop=True,
                    )
                else:
                    proj_group(*g)

            # Remaining 20 groups (u j1 first — needed by block (1,1) —
            # then j2, j3) spread one per vert-half through the main
            # stream; DVE (their binarize engine) drains them with slack
            # before first use.
            SPREAD = (
                [(True, 1, a) for a in range(KC // 2)]
                + [(True, 2, a) for a in range(KC // 2)]
                + [(False, 2, a) for a in range(KC // 2)]
                + [(True, 3, a) for a in range(KC // 2)]
                + [(False, 3, a) for a in range(KC // 2)]
            )

            # Block order (a = u j-block, b = v j-block) grows the set of
            # needed code blocks slowly so binarize is never on the
            # critical path.
            BLOCKS = [
                (0, 0), (0, 1), (1, 1), (1, 0),
                (2, 0), (2, 1), (0, 2), (1, 2),
                (2, 2), (3, 0), (3, 1), (3, 2),
                (0, 3), (1, 3), (2, 3), (3, 3),
            ]
            nvh = 2 * len(BLOCKS)
            vhi = 0
            si = 0
            for (a, b) in BLOCKS:
                for pair in range(2):
                    # Alternate output DMA queues, but route the tail to
                    # sync (HWDGE) so the gpsimd SWDGE ring drains early.
                    dmaq = nc.sync if (vhi % 2 == 0 or vhi >= nvh - 6) else nc.gpsimd
                    if si < len(SPREAD):
                        proj_group(*SPREAD[si])
                        si += 1
                    # fine (per-bank Sin) also for the first two halves:
                    # issuing bank0's Sin ~0.9us earlier shortens the mp
                    # recycle wait while ACT is otherwise idle.
                    vert_half(a, pair, b, dmaq, fine=(vhi >= nvh - 2 or vhi < 2))
                    vhi += 1

    nc.compile()
    _BUILD_CACHE[scale] = nc
    return nc


def _r22(a):
    """Round fp32 to 13 mantissa bits (nearest-even) — the PE's fp32r
    path truncates operands to ~FP22, so pre-rounding on the host turns
    that truncation into a no-op and halves the effective input error."""
    u = a.view(np.uint32)
    lsb = (u >> np.uint32(10)) & np.uint32(1)
    return ((u + np.uint32(0x1FF) + lsb) & np.uint32(0xFFFFFC00)).view(np.float32)


def _in_maps(emb1, emb2, r):
    rt = _r22(np.ascontiguousarray(r.T))
    e1t = _r22(np.ascontiguousarray(emb1.T))
    e2t = _r22(np.ascontiguousarray(emb2.T))
    maps = []
    for k in range(8):
        a, b = k // G2, k % G2
        maps.append(
            {
                "e1": np.ascontiguousarray(e1t[:, a * M1 : (a + 1) * M1]),
                "e2": np.ascontiguousarray(e2t[:, b * M2 : (b + 1) * M2]),
                "rr": rt,
            }
        )
    return maps


def _install_profile_hook():
    """The agent image's antenv lacks axon_hooks; synthesize it so
    run_bass_kernel_spmd(trace=True) can reach the NTFF profiler."""
    import types

    if "antenv.axon_hooks" in sys.modules:
        return
    try:
        from trn_agent_boot.trn_boot import _ntff_profile_via_ctypes

        hook = _ntff_profile_via_ctypes("/opt/axon/libaxon_pjrt.so")
        mod = types.ModuleType("antenv.axon_hooks")
        mod.get_axon_ntff_profile_hook = lambda: hook
        sys.modules["antenv.axon_hooks"] = mod

        from concourse import bass_utils as _bu

        _orig_upload = _bu.upload_artifacts

        def _safe_upload(tmpdir):
            try:
                return _orig_upload(tmpdir)
            except Exception as e:  # no bucket access in this container
                return f"upload-skipped: {e}"

        _bu.upload_artifacts = _safe_upload
    except Exception:
        pass


def kernel(emb1, emb2, r, pi, _trace=False, _tmpdir=None):
    emb1 = np.asarray(emb1, dtype=np.float32)
    emb2 = np.asarray(emb2, dtype=np.float32)
    r = np.asarray(r, dtype=np.float32)
    # u codes +/-1, v codes +/-0.5: psum P = 0.5 * s_u.s_v, out = sin((pi/B)*P)
    scale = float(np.asarray(pi).reshape(-1)[0]) / B

    nc = _build(scale)
    if _trace:
        _install_profile_hook()
    try:
        res = run_bass_kernel_spmd(
            nc, _in_maps(emb1, emb2, r), list(range(8)), trace=_trace, tmpdir=_tmpdir
        )
    except ModuleNotFoundError:
        res = run_bass_kernel_spmd(nc, _in_maps(emb1, emb2, r), list(range(8)))

    full = np.empty((N1, N2), dtype=np.float32)
    for k in range(8):
        a, b = k // G2, k % G2
        full[a * M1 : (a + 1) * M1, b * M2 : (b + 1) * M2] = np.asarray(
            res.results[k]["out"]
        ).astype(np.float32)
    if _trace:
        kernel._last_exec_time_ns = res.exec_time_ns
    return full



# revision 42
# speedup vs baseline: 1.0005x; 1.0005x over previous
"""LSH cosine-of-Hamming retrieval kernel for 8 trn2 NeuronCores.

Math: reference computes cos((pi/d) * hamming(u, v)) for binary LSH codes
u = (emb1 @ r.T > 0), v = (emb2 @ r.T > 0), d = 1024 bits.
With +/-1 sign codes s_u = 2u-1, s_v = 2v-1:
    hamming = (d - s_u . s_v) / 2
    cos((pi/d) * hamming) = sin((pi/2d) * s_u.s_v)
u codes are stored as +/-1 fp8 (ACT Sign), v codes as +/-0.5 fp8 (DVE
is_gt/subtract), so psum P = 0.5 * s_u.s_v and out = sin((pi/d) * P).

Projection runs in ONE fp32r pass (the PE truncates operands to ~FP22
and products to similar precision; inputs are pre-rounded to 13
mantissa bits on the host so the input truncation is a no-op, leaving
~6e-5 sign-flip rate -> 9.6e-3 rel err, inside the 2e-2 budget). This
replaces the 3-pass bf16 hi/lo split projection: 64 instead of 192
projection matmuls (fp32r streams ~1.7 cycles/row vs bf16's 1.0, so
projection PE time drops 41us -> ~24us).

The binarize of the projection psum (4M fp32 values) is the side
constraint: GPSIMD cannot read PSUM and DVE 2x perf modes need 16-bit
or SBUF sources, so only DVE and ACT can drain codes, at 1 elem/cycle
with a ~0.3-0.5us fixed cost per instruction. Three consequences shape
the schedule:
- binarize in 2-bank groups (one instruction per two 128-bit chunks)
  to amortize the fixed cost (~16us/engine instead of ~22-26);
- code magnitude only needs to be uniform per (tensor, 512-wide
  j-block), because each main psum bank contracts exactly one u
  j-block against one v j-block — the per-half Sin scale absorbs the
  magnitude product. So u j0 is ACT Sign (+/-1) and everything else is
  DVE is_gt/sub (+/-0.5), keeping ACT free for the Sin stream;
- only u j0 + v j0 + v j1 are projected upfront; the remaining 20
  groups trail one-per-half through the main stream (DVE drains them
  with slack before first use, per the block order below).

Main phase: fp8 DoubleRow "vertical halves" (two adjacent 128-row
blocks x one 512-col quarter, one psum bank each), ordered by (u j,
v j) block so the needed-code set grows slowly; Sin on ACT with the
per-half scale, output DMAs alternating sync/gpsimd queues (tail on
sync so the SWDGE ring drains early). The first/last two halves use
per-bank Sin/DMA quanta to shorten recycle waits and the drain.

The PE clock (HAM) re-throttles to 1.2GHz after ~3.4us of accumulated
idle in its window, costing ~5-7us per event: 12 warm-up matmuls cover
the DMA-latency head, warm fillers plug the known arrival-wait sites
in the upfront phase, and input DMAs spread over three queues (r +
late e1 on sync, early e1 on scalar, e2 on gpsimd) keep per-queue
chains short so no early arrival lands late enough to stall the PE
past the window. Do NOT interleave main matmuls chunk-by-chunk into
the projection stream (chunk-pipelined opening): matmuls parked on
code semaphores while weight loads stream past trip the LDWEIGHTS
clobber below and intermittently corrupt whole output quadrants.

NOTE: do NOT dedupe back-to-back identical LDWEIGHTS here - on this
runtime the PE's reorder window executes pulled-ahead weight loads
while early matmuls are parked on unsatisfied code semaphores, and the
second matmul of a pair then runs with clobbered weights (reproducible
NaN quadrants in the first scheduled halves).

Sharding (2x4 grid over 8 cores): core k computes the [2048, 2048]
output block for emb1 rows [(k//4)*2048...] x emb2 rows [(k%4)*2048...];
r is replicated (collectives cost ~60us fixed here - not worth it).
Host prep is layout-only: transpose + fp22 pre-round.
"""

import sys

sys.path.insert(0, "/opt/trn_rl_repo")

import numpy as np

import concourse.bacc as bacc
import concourse.tile as tile
from concourse import mybir
from concourse.bass_utils import run_bass_kernel_spmd

N1, N2, D, B = 4096, 8192, 128, 1024  # emb1 rows, emb2 rows, dim, num_bits
G1, G2 = 2, 4
M1, M2 = N1 // G1, N2 // G2  # 2048 x 2048 output block per core
KC = B // 128  # 8 bit-chunks of 128
RW = 512  # projection row-chunk width
NW = 512  # main matmul psum tile width

_BUILD_CACHE = {}


def _build(scale: float):
    if scale in _BUILD_CACHE:
        return _BUILD_CACHE[scale]
    nc = bacc.Bacc("TRN2", target_bir_lowering=False, debug=False)
    f32 = mybir.dt.float32
    f32r = mybir.dt.float32r
    bf16 = mybir.dt.bfloat16
    fp8 = mybir.dt.float8e4
    A = mybir.AluOpType
    AF = mybir.ActivationFunctionType

    e1 = nc.declare_dram_parameter("e1", [D, M1], f32r, isOutput=False)
    e2 = nc.declare_dram_parameter("e2", [D, M2], f32r, isOutput=False)
    rr = nc.declare_dram_parameter("rr", [D, B], f32r, isOutput=False)
    out = nc.declare_dram_parameter("out", [M1, M2], bf16, isOutput=True)

    with tile.TileContext(nc) as tc:
        with (
            tc.tile_pool(name="const", bufs=1) as const_pool,
            tc.tile_pool(name="outs", bufs=6) as out_pool,
            tc.tile_pool(name="pproj", bufs=2, space="PSUM") as pp,
            tc.tile_pool(name="pmain", bufs=2, space="PSUM") as mp,
        ):
            r_sb = const_pool.tile([D, B], f32r)
            e1_sb = const_pool.tile([D, M1], f32r)
            e2_sb = const_pool.tile([D, M2], f32r)
            ut = const_pool.tile([128, KC, M1], fp8)
            vt = const_pool.tile([128, KC, M2], fp8)

            # Warm-up operand: read a corner of the (not-yet-written)
            # code tile — the psum result is discarded, so garbage is
            # fine, and skipping a memset lets the gpsimd queue issue
            # e2 j0 ~0.5us earlier and the warm-up start unconditionally.
            # The u j3 corner is binarized ~20us after the last filler
            # read, so the write-after-read order costs nothing.
            warm = ut

            # Input DMAs across three queues in parallel (issue-to-land
            # latency is ~4.8us fixed, transfers ~0.75us per 256KB piece):
            # sync carries r (the gate for every projection) + late e1,
            # scalar carries early e1, gpsimd (SWDGE: jittier first
            # landing, but idle until the output DMAs at ~21us) carries
            # all of e2. Short per-queue chains land the early arrivals
            # sooner and accumulate less jitter — a +3us-late early
            # arrival stalls the PE long enough to re-throttle the clock.
            nc.sync.dma_start(r_sb[:, 0:512], rr[:, 0:512])
            nc.scalar.dma_start(e1_sb[:, 0:RW], e1[:, 0:RW])
            nc.gpsimd.dma_start(e2_sb[:, 0:RW], e2[:, 0:RW])
            nc.sync.dma_start(r_sb[:, 512:], rr[:, 512:])
            nc.scalar.dma_start(e1_sb[:, RW : 2 * RW], e1[:, RW : 2 * RW])
            for j in (1, 2, 3):
                nc.gpsimd.dma_start(
                    e2_sb[:, j * RW : (j + 1) * RW], e2[:, j * RW : (j + 1) * RW]
                )
            for j in (2, 3):
                nc.sync.dma_start(
                    e1_sb[:, j * RW : (j + 1) * RW], e1[:, j * RW : (j + 1) * RW]
                )

            # HAM warm-up: the PE clock ramps to 2.4GHz only after ~3.5us
            # of sustained activity (trace: proj ran at 1.2GHz until
            # t~20us with only 8 warmups + a gap). Burn enough dummy
            # matmuls to (a) cover the >=3.5us busy window and (b) keep
            # the PE busy until the first input DMAs land (~12us), so
            # projection starts on a warm clock with no PE idle gap.
            for _ in range(13):
                wps = pp.tile([128, 2, RW], f32, name="pjtile", tag="pj")
                nc.tensor.matmul(
                    wps[:, 0, :],
                    warm[:, KC - 1, M1 - 128 : M1],
                    warm[:, KC - 1, M1 - RW : M1],
                    start=True,
                    stop=True,
                )

            # Engine assignment per (tensor, j-block): a code block's
            # magnitude only needs to be uniform within its own j-block,
            # because each main psum bank reads exactly one u j-block and
            # one v j-block — the per-half Sin scale absorbs the product
            # of magnitudes. ACT Sign gives +/-1, DVE is_gt/sub +/-0.5.
            # Only u j0 goes to ACT (needed first, before ACT has Sin
            # work); everything else goes to DVE — this keeps ACT free to
            # stream Sins with no binarize bursts delaying the main psum
            # recycle.
            def on_act(is_u, j):
                return is_u and j == 0

            def proj_group(is_u, j, a, in_mp=False):
                # Two 128-bit chunks (c=2a, 2a+1) into one 2-bank psum
                # tile, drained by a single binarize instruction: ACT has
                # ~494ns fixed cost per instruction, so 2-bank groups cut
                # the binarize engine time from ~22us to ~16us per engine
                # (the projection phase is binarize-bound, not PE-bound).
                # During the upfront phase the main pool is idle, so
                # alternating groups across both pools doubles the psum
                # recycle depth (binarize latency hides behind 4 tiles).
                src = e1_sb if is_u else e2_sb
                dst = ut if is_u else vt
                sl = slice(j * RW, (j + 1) * RW)
                if in_mp:
                    ps = mp.tile([128, 2, RW], f32, name="pmtile", tag="pm")
                else:
                    ps = pp.tile([128, 2, RW], f32, name="pjtile", tag="pj")
                for i in range(2):
                    cs = slice((2 * a + i) * 128, (2 * a + i + 1) * 128)
                    nc.tensor.matmul(
                        ps[:, i, :], r_sb[:, cs], src[:, sl], start=True, stop=True
                    )
                if on_act(is_u, j):
                    nc.scalar.activation(dst[:, 2 * a : 2 * a + 2, sl], ps[:], AF.Sign)
                else:
                    nc.vector.tensor_scalar(
                        dst[:, 2 * a : 2 * a + 2, sl], ps[:], 0.0, 0.5, A.is_gt, A.subtract
                    )

            def vert_half(a, pair, b, dmaq, fine=False):
                # "Vertical half": two adjacent 128-row blocks x one 512
                # column quarter, one psum bank per row block. The second
                # bank's matmuls only need the same (u j, v j) codes, so a
                # vert-half depends on exactly one u j-block and one v
                # j-block — main work can start as soon as u j0 + v j0
                # codes exist.
                ps = mp.tile([128, 2, NW], f32, name="pmtile", tag="pm")
                ot = out_pool.tile([128, 2 * NW], bf16)
                ns = slice(b * NW, (b + 1) * NW)
                # psum = u_mag * v_mag * (s_u . s_v); Sin(scale * psum)
                # needs scale = (pi/2d) / (u_mag * v_mag); ACT-binarized
                # blocks are +/-1, DVE ones +/-0.5.
                um = 1.0 if on_act(True, a) else 0.5
                vm = 1.0 if on_act(False, b) else 0.5
                sc = scale * 0.5 / (um * vm)
                for mi in range(2):
                    m = 4 * a + 2 * pair + mi
                    ms = slice(m * 128, (m + 1) * 128)
                    for s in range(KC // 2):
                        nc.tensor.matmul(
                            ps[:, mi, :],
                            ut[:, 2 * s : 2 * s + 2, ms],
                            vt[:, 2 * s : 2 * s + 2, ns],
                            start=(s == 0),
                            stop=(s == KC // 2 - 1),
                            perf_mode=mybir.MatmulPerfMode.DoubleRow,
                        )
                    if fine:
                        # last vert-halves: per-bank Sin/DMA so bank0's
                        # drain overlaps bank1's matmuls
                        nc.scalar.activation(
                            ot[:, mi * NW : (mi + 1) * NW], ps[:, mi, :], AF.Sin, scale=sc
                        )
                        dmaq.dma_start(out[ms, ns], ot[:, mi * NW : (mi + 1) * NW])
                if not fine:
                    nc.scalar.activation(ot[:], ps[:], AF.Sin, scale=sc)
                    for mi in range(2):
                        m = 4 * a + 2 * pair + mi
                        ms = slice(m * 128, (m + 1) * 128)
                        dmaq.dma_start(out[ms, ns], ot[:, mi * NW : (mi + 1) * NW])

            # Upfront projection: only what the first blocks need — u j0
            # (ACT), v j0 and v j1 (DVE), interleaved in DMA-arrival
            # order. The first vert-half waits only for u j0 + v j0.
            # None: warm filler matmuls at the known DMA-arrival stall
            # sites (e2 j0, r tail) — a 1-2.5us PE idle there can cross
            # the HAM MID window and re-throttle the clock to 1.2GHz for
            # ~7us; the fillers substitute busy-work for that idle.
            wfill = mp.tile([128, 2, RW], f32, name="pmtile", tag="pm")
            UP = [
                (True, 0, 0), None, (True, 0, 1), None, None, (False, 0, 0),
                (True, 0, 2), None, (False, 0, 1), (True, 0, 3), None,
                (False, 0, 2), (False, 0, 3), None, (False, 1, 0),
                (False, 1, 1), (False, 1, 2), (False, 1, 3),
            ]
            for g in UP:
                if g is None:
                    nc.tensor.matmul(
                        wfill[:, 0, :],
                        warm[:, KC - 1, M1 - 128 : M1],
                        warm[:, KC - 1, M1 - RW : M1],
                        start=True,
                        stop=True,
                    )
                else:
                    proj_group(*g)

            # Remaining 20 groups (u j1 first — needed by block (1,1) —
            # then j2, j3) spread one per vert-half through the main
            # stream; DVE (their binarize engine) drains them with slack
            # before first use.
            SPREAD = (
                [(True, 1, a) for a in range(KC // 2)]
                + [(True, 2, a) for a in range(KC // 2)]
                + [(False, 2, a) for a in range(KC // 2)]
                + [(True, 3, a) for a in range(KC // 2)]
                + [(False, 3, a) for a in range(KC // 2)]
            )

            # Block order (a = u j-block, b = v j-block) grows the set of
            # needed code blocks slowly so binarize is never on the
            # critical path.
            BLOCKS = [
                (0, 0), (0, 1), (1, 1), (1, 0),
                (2, 0), (2, 1), (0, 2), (1, 2),
                (2, 2), (3, 0), (3, 1), (3, 2),
                (0, 3), (1, 3), (2, 3), (3, 3),
            ]
            nvh = 2 * len(BLOCKS)
            vhi = 0
            si = 0
            for (a, b) in BLOCKS:
                for pair in range(2):
                    # Alternate output DMA queues, but route the tail to
                    # sync (HWDGE) so the gpsimd SWDGE ring drains early.
                    dmaq = nc.sync if (vhi % 2 == 0 or vhi >= nvh - 6) else nc.gpsimd
                    if si < len(SPREAD):
                        proj_group(*SPREAD[si])
                        si += 1
                    # fine (per-bank Sin) also for the first two halves:
                    # issuing bank0's Sin ~0.9us earlier shortens the mp
                    # recycle wait while ACT is otherwise idle.
                    vert_half(a, pair, b, dmaq, fine=(vhi >= nvh - 2 or vhi < 2))
                    vhi += 1

    nc.compile()
    _BUILD_CACHE[scale] = nc
    return nc


def _r22(a):
    """Round fp32 to 13 mantissa bits (nearest-even) — the PE's fp32r
    path truncates operands to ~FP22, so pre-rounding on the host turns
    that truncation into a no-op and halves the effective input error."""
    u = a.view(np.uint32)
    lsb = (u >> np.uint32(10)) & np.uint32(1)
    return ((u + np.uint32(0x1FF) + lsb) & np.uint32(0xFFFFFC00)).view(np.float32)


def _in_maps(emb1, emb2, r):
    rt = _r22(np.ascontiguousarray(r.T))
    e1t = _r22(np.ascontiguousarray(emb1.T))
    e2t = _r22(np.ascontiguousarray(emb2.T))
    maps = []
    for k in range(8):
        a, b = k // G2, k % G2
        maps.append(
            {
                "e1": np.ascontiguousarray(e1t[:, a * M1 : (a + 1) * M1]),
                "e2": np.ascontiguousarray(e2t[:, b * M2 : (b + 1) * M2]),
                "rr": rt,
            }
        )
    return maps


def _install_profile_hook():
    """The agent image's antenv lacks axon_hooks; synthesize it so
    run_bass_kernel_spmd(trace=True) can reach the NTFF profiler."""
    import types

    if "antenv.axon_hooks" in sys.modules:
        return
    try:
        from trn_agent_boot.trn_boot import _ntff_profile_via_ctypes

        hook = _ntff_profile_via_ctypes("/opt/axon/libaxon_pjrt.so")
        mod = types.ModuleType("antenv.axon_hooks")
        mod.get_axon_ntff_profile_hook = lambda: hook
        sys.modules["antenv.axon_hooks"] = mod

        from concourse import bass_utils as _bu

        _orig_upload = _bu.upload_artifacts

        def _safe_upload(tmpdir):
            try:
                return _orig_upload(tmpdir)
            except Exception as e:  # no bucket access in this container
                return f"upload-skipped: {e}"

        _bu.upload_artifacts = _safe_upload
    except Exception:
        pass


def kernel(emb1, emb2, r, pi, _trace=False, _tmpdir=None):
    emb1 = np.asarray(emb1, dtype=np.float32)
    emb2 = np.asarray(emb2, dtype=np.float32)
    r = np.asarray(r, dtype=np.float32)
    # u codes +/-1, v codes +/-0.5: psum P = 0.5 * s_u.s_v, out = sin((pi/B)*P)
    scale = float(np.asarray(pi).reshape(-1)[0]) / B

    nc = _build(scale)
    if _trace:
        _install_profile_hook()
    try:
        res = run_bass_kernel_spmd(
            nc, _in_maps(emb1, emb2, r), list(range(8)), trace=_trace, tmpdir=_tmpdir
        )
    except ModuleNotFoundError:
        res = run_bass_kernel_spmd(nc, _in_maps(emb1, emb2, r), list(range(8)))

    full = np.empty((N1, N2), dtype=np.float32)
    for k in range(8):
        a, b = k // G2, k % G2
        full[a * M1 : (a + 1) * M1, b * M2 : (b + 1) * M2] = np.asarray(
            res.results[k]["out"]
        ).astype(np.float32)
    if _trace:
        kernel._last_exec_time_ns = res.exec_time_ns
    return full



# revision 43
# speedup vs baseline: 1.0319x; 1.0314x over previous
"""LSH cosine-of-Hamming retrieval kernel for 8 trn2 NeuronCores.

Math: reference computes cos((pi/d) * hamming(u, v)) for binary LSH codes
u = (emb1 @ r.T > 0), v = (emb2 @ r.T > 0), d = 1024 bits.
With +/-1 sign codes s_u = 2u-1, s_v = 2v-1:
    hamming = (d - s_u . s_v) / 2
    cos((pi/d) * hamming) = sin((pi/2d) * s_u.s_v)
u codes are stored as +/-1 fp8 (ACT Sign), v codes as +/-0.5 fp8 (DVE
is_gt/subtract), so psum P = 0.5 * s_u.s_v and out = sin((pi/d) * P).

Projection runs in ONE fp32r pass (the PE truncates operands to ~FP22
and products to similar precision; inputs are pre-rounded to 13
mantissa bits on the host so the input truncation is a no-op, leaving
~6e-5 sign-flip rate -> 9.6e-3 rel err, inside the 2e-2 budget). This
replaces the 3-pass bf16 hi/lo split projection: 64 instead of 192
projection matmuls (fp32r streams ~1.7 cycles/row vs bf16's 1.0, so
projection PE time drops 41us -> ~24us).

The binarize of the projection psum (4M fp32 values) is the side
constraint: GPSIMD cannot read PSUM and DVE 2x perf modes need 16-bit
or SBUF sources, so only DVE and ACT can drain codes, at 1 elem/cycle
with a ~0.3-0.5us fixed cost per instruction. Three consequences shape
the schedule:
- binarize in 2-bank groups (one instruction per two 128-bit chunks)
  to amortize the fixed cost (~16us/engine instead of ~22-26);
- code magnitude only needs to be uniform per (tensor, 512-wide
  j-block), because each main psum bank contracts exactly one u
  j-block against one v j-block — the per-half Sin scale absorbs the
  magnitude product. So u j0 is ACT Sign (+/-1) and everything else is
  DVE is_gt/sub (+/-0.5), keeping ACT free for the Sin stream;
- only u j0 + v j0 + v j1 are projected upfront; the remaining 20
  groups trail one-per-half through the main stream (DVE drains them
  with slack before first use, per the block order below).

Main phase: fp8 DoubleRow "vertical halves" (two adjacent 128-row
blocks x one 512-col quarter, one psum bank each), ordered by (u j,
v j) block so the needed-code set grows slowly; Sin on ACT with the
per-half scale, output DMAs alternating sync/gpsimd queues (tail on
sync so the SWDGE ring drains early). The first/last two halves use
per-bank Sin/DMA quanta to shorten recycle waits and the drain.

The PE clock (HAM) re-throttles to 1.2GHz after ~3.4us of accumulated
idle in its window, costing ~5-7us per event: 12 warm-up matmuls cover
the DMA-latency head, warm fillers plug the known arrival-wait sites
in the upfront phase, and input DMAs spread over three queues (r +
late e1 on sync, early e1 on scalar, e2 on gpsimd) keep per-queue
chains short so no early arrival lands late enough to stall the PE
past the window. Do NOT interleave main matmuls chunk-by-chunk into
the projection stream (chunk-pipelined opening): matmuls parked on
code semaphores while weight loads stream past trip the LDWEIGHTS
clobber below and intermittently corrupt whole output quadrants.

NOTE: do NOT dedupe back-to-back identical LDWEIGHTS here - on this
runtime the PE's reorder window executes pulled-ahead weight loads
while early matmuls are parked on unsatisfied code semaphores, and the
second matmul of a pair then runs with clobbered weights (reproducible
NaN quadrants in the first scheduled halves).

Sharding (2x4 grid over 8 cores): core k computes the [2048, 2048]
output block for emb1 rows [(k//4)*2048...] x emb2 rows [(k%4)*2048...];
r is replicated (collectives cost ~60us fixed here - not worth it).
Host prep is layout-only: transpose + fp22 pre-round.
"""

import sys

sys.path.insert(0, "/opt/trn_rl_repo")

import numpy as np

import concourse.bacc as bacc
import concourse.tile as tile
from concourse import mybir
from concourse.bass_utils import run_bass_kernel_spmd

N1, N2, D, B = 4096, 8192, 128, 1024  # emb1 rows, emb2 rows, dim, num_bits
G1, G2 = 2, 4
M1, M2 = N1 // G1, N2 // G2  # 2048 x 2048 output block per core
KC = B // 128  # 8 bit-chunks of 128
RW = 512  # projection row-chunk width
NW = 512  # main matmul psum tile width

_BUILD_CACHE = {}


def _build(scale: float):
    if scale in _BUILD_CACHE:
        return _BUILD_CACHE[scale]
    nc = bacc.Bacc("TRN2", target_bir_lowering=False, debug=False)
    f32 = mybir.dt.float32
    f32r = mybir.dt.float32r
    bf16 = mybir.dt.bfloat16
    fp8 = mybir.dt.float8e4
    A = mybir.AluOpType
    AF = mybir.ActivationFunctionType

    e1 = nc.declare_dram_parameter("e1", [D, M1], f32r, isOutput=False)
    e2 = nc.declare_dram_parameter("e2", [D, M2], f32r, isOutput=False)
    rr = nc.declare_dram_parameter("rr", [D, B], f32r, isOutput=False)
    out = nc.declare_dram_parameter("out", [M1, M2], bf16, isOutput=True)

    with tile.TileContext(nc) as tc:
        with (
            tc.tile_pool(name="const", bufs=1) as const_pool,
            tc.tile_pool(name="outs", bufs=6) as out_pool,
            tc.tile_pool(name="pproj", bufs=2, space="PSUM") as pp,
            tc.tile_pool(name="pmain", bufs=2, space="PSUM") as mp,
        ):
            r_sb = const_pool.tile([D, B], f32r)
            e1_sb = const_pool.tile([D, M1], f32r)
            e2_sb = const_pool.tile([D, M2], f32r)
            ut = const_pool.tile([128, KC, M1], fp8)
            vt = const_pool.tile([128, KC, M2], fp8)

            # Warm-up operand: read a corner of the (not-yet-written)
            # code tile — the psum result is discarded, so garbage is
            # fine, and skipping a memset lets the gpsimd queue issue
            # e2 j0 ~0.5us earlier and the warm-up start unconditionally.
            # The u j3 corner is binarized ~20us after the last filler
            # read, so the write-after-read order costs nothing.
            warm = ut

            # Input DMAs across three queues in parallel (issue-to-land
            # latency is ~4.8us fixed, transfers ~0.75us per 256KB piece):
            # sync carries r (the gate for every projection) + late e1,
            # scalar carries early e1, gpsimd (SWDGE: jittier first
            # landing, but idle until the output DMAs at ~21us) carries
            # all of e2. Short per-queue chains land the early arrivals
            # sooner and accumulate less jitter — a +3us-late early
            # arrival stalls the PE long enough to re-throttle the clock.
            nc.sync.dma_start(r_sb[:, 0:512], rr[:, 0:512])
            nc.scalar.dma_start(e1_sb[:, 0:RW], e1[:, 0:RW])
            nc.gpsimd.dma_start(e2_sb[:, 0:RW], e2[:, 0:RW])
            nc.sync.dma_start(r_sb[:, 512:], rr[:, 512:])
            nc.scalar.dma_start(e1_sb[:, RW : 2 * RW], e1[:, RW : 2 * RW])
            for j in (1, 2, 3):
                nc.gpsimd.dma_start(
                    e2_sb[:, j * RW : (j + 1) * RW], e2[:, j * RW : (j + 1) * RW]
                )
            for j in (2, 3):
                nc.sync.dma_start(
                    e1_sb[:, j * RW : (j + 1) * RW], e1[:, j * RW : (j + 1) * RW]
                )

            # HAM warm-up: the PE clock ramps to 2.4GHz only after ~3.5us
            # of sustained activity (trace: proj ran at 1.2GHz until
            # t~20us with only 8 warmups + a gap). Burn enough dummy
            # matmuls to (a) cover the >=3.5us busy window and (b) keep
            # the PE busy until the first input DMAs land (~12us), so
            # projection starts on a warm clock with no PE idle gap.
            for _ in range(13):
                wps = pp.tile([128, 2, RW], f32, name="pjtile", tag="pj")
                nc.tensor.matmul(
                    wps[:, 0, :],
                    warm[:, KC - 1, M1 - 128 : M1],
                    warm[:, KC - 1, M1 - RW : M1],
                    start=True,
                    stop=True,
                )

            # Engine assignment per (tensor, j-block): a code block's
            # magnitude only needs to be uniform within its own j-block,
            # because each main psum bank reads exactly one u j-block and
            # one v j-block — the per-half Sin scale absorbs the product
            # of magnitudes. ACT Sign gives +/-1, DVE is_gt/sub +/-0.5.
            # Only u j0 goes to ACT (needed first, before ACT has Sin
            # work); everything else goes to DVE — this keeps ACT free to
            # stream Sins with no binarize bursts delaying the main psum
            # recycle.
            def on_act(is_u, j):
                return is_u and j == 0

            def proj_group(is_u, j, a, in_mp=False):
                # Two 128-bit chunks (c=2a, 2a+1) into one 2-bank psum
                # tile, drained by a single binarize instruction: ACT has
                # ~494ns fixed cost per instruction, so 2-bank groups cut
                # the binarize engine time from ~22us to ~16us per engine
                # (the projection phase is binarize-bound, not PE-bound).
                # During the upfront phase the main pool is idle, so
                # alternating groups across both pools doubles the psum
                # recycle depth (binarize latency hides behind 4 tiles).
                src = e1_sb if is_u else e2_sb
                dst = ut if is_u else vt
                sl = slice(j * RW, (j + 1) * RW)
                if in_mp:
                    ps = mp.tile([128, 2, RW], f32, name="pmtile", tag="pm")
                else:
                    ps = pp.tile([128, 2, RW], f32, name="pjtile", tag="pj")
                for i in range(2):
                    cs = slice((2 * a + i) * 128, (2 * a + i + 1) * 128)
                    nc.tensor.matmul(
                        ps[:, i, :], r_sb[:, cs], src[:, sl], start=True, stop=True
                    )
                if on_act(is_u, j):
                    nc.scalar.activation(dst[:, 2 * a : 2 * a + 2, sl], ps[:], AF.Sign)
                else:
                    nc.vector.tensor_scalar(
                        dst[:, 2 * a : 2 * a + 2, sl], ps[:], 0.0, 0.5, A.is_gt, A.subtract
                    )

            def vert_half(a, pair, b, dmaq, fine=False):
                # "Vertical half": two adjacent 128-row blocks x one 512
                # column quarter, one psum bank per row block. The second
                # bank's matmuls only need the same (u j, v j) codes, so a
                # vert-half depends on exactly one u j-block and one v
                # j-block — main work can start as soon as u j0 + v j0
                # codes exist.
                ps = mp.tile([128, 2, NW], f32, name="pmtile", tag="pm")
                ot = out_pool.tile([128, 2 * NW], bf16)
                ns = slice(b * NW, (b + 1) * NW)
                # psum = u_mag * v_mag * (s_u . s_v); Sin(scale * psum)
                # needs scale = (pi/2d) / (u_mag * v_mag); ACT-binarized
                # blocks are +/-1, DVE ones +/-0.5.
                um = 1.0 if on_act(True, a) else 0.5
                vm = 1.0 if on_act(False, b) else 0.5
                sc = scale * 0.5 / (um * vm)
                for mi in range(2):
                    m = 4 * a + 2 * pair + mi
                    ms = slice(m * 128, (m + 1) * 128)
                    for s in range(KC // 2):
                        nc.tensor.matmul(
                            ps[:, mi, :],
                            ut[:, 2 * s : 2 * s + 2, ms],
                            vt[:, 2 * s : 2 * s + 2, ns],
                            start=(s == 0),
                            stop=(s == KC // 2 - 1),
                            perf_mode=mybir.MatmulPerfMode.DoubleRow,
                        )
                    if fine:
                        # last vert-halves: per-bank Sin/DMA so bank0's
                        # drain overlaps bank1's matmuls
                        nc.scalar.activation(
                            ot[:, mi * NW : (mi + 1) * NW], ps[:, mi, :], AF.Sin, scale=sc
                        )
                        dmaq.dma_start(out[ms, ns], ot[:, mi * NW : (mi + 1) * NW])
                if not fine:
                    nc.scalar.activation(ot[:], ps[:], AF.Sin, scale=sc)
                    for mi in range(2):
                        m = 4 * a + 2 * pair + mi
                        ms = slice(m * 128, (m + 1) * 128)
                        dmaq.dma_start(out[ms, ns], ot[:, mi * NW : (mi + 1) * NW])

            # Upfront projection: only what the first blocks need — u j0
            # (ACT), v j0 and v j1 (DVE), interleaved in DMA-arrival
            # order. The first vert-half waits only for u j0 + v j0.
            # None: warm filler matmuls at the known DMA-arrival stall
            # sites (e2 j0, r tail) — a 1-2.5us PE idle there can cross
            # the HAM MID window and re-throttle the clock to 1.2GHz for
            # ~7us; the fillers substitute busy-work for that idle.
            wfill = mp.tile([128, 2, RW], f32, name="pmtile", tag="pm")
            UP = [
                (True, 0, 0), None, (True, 0, 1), None, None, (False, 0, 0),
                (True, 0, 2), None, (False, 0, 1), (True, 0, 3), None,
                (False, 0, 2), (False, 0, 3), None, (False, 1, 0),
                (False, 1, 1), (False, 1, 2), (False, 1, 3),
            ]
            for g in UP:
                if g is not None and (not g[0]) and g[1] == 0:
                    # v j0 borrows the idle main pool: its DVE binarize
                    # completes before the first vert-half reclaims those
                    # buffers, and the loan doubles the upfront psum
                    # recycle depth.
                    proj_group(*g, in_mp=True)
                    continue
                if g is None:
                    nc.tensor.matmul(
                        wfill[:, 0, :],
                        warm[:, KC - 1, M1 - 128 : M1],
                        warm[:, KC - 1, M1 - RW : M1],
                        start=True,
                        stop=True,
                    )
                else:
                    proj_group(*g)

            # Remaining 20 groups (u j1 first — needed by block (1,1) —
            # then j2, j3) spread one per vert-half through the main
            # stream; DVE (their binarize engine) drains them with slack
            # before first use.
            SPREAD = (
                [(True, 1, a) for a in range(KC // 2)]
                + [(True, 2, a) for a in range(KC // 2)]
                + [(False, 2, a) for a in range(KC // 2)]
                + [(True, 3, a) for a in range(KC // 2)]
                + [(False, 3, a) for a in range(KC // 2)]
            )

            # Block order (a = u j-block, b = v j-block) grows the set of
            # needed code blocks slowly so binarize is never on the
            # critical path.
            BLOCKS = [
                (0, 0), (0, 1), (1, 1), (1, 0),
                (2, 0), (2, 1), (0, 2), (1, 2),
                (2, 2), (3, 0), (3, 1), (3, 2),
                (0, 3), (1, 3), (2, 3), (3, 3),
            ]
            nvh = 2 * len(BLOCKS)
            vhi = 0
            si = 0
            for (a, b) in BLOCKS:
                for pair in range(2):
                    # Alternate output DMA queues, but route the tail to
                    # sync (HWDGE) so the gpsimd SWDGE ring drains early.
                    dmaq = nc.sync if (vhi % 2 == 0 or vhi >= nvh - 6) else nc.gpsimd
                    if si < len(SPREAD):
                        proj_group(*SPREAD[si])
                        si += 1
                    # fine (per-bank Sin) also for the first two halves:
                    # issuing bank0's Sin ~0.9us earlier shortens the mp
                    # recycle wait while ACT is otherwise idle.
                    vert_half(a, pair, b, dmaq, fine=(vhi >= nvh - 2 or vhi < 2))
                    vhi += 1

    nc.compile()
    _BUILD_CACHE[scale] = nc
    return nc


def _r22(a):
    """Round fp32 to 13 mantissa bits (nearest-even) — the PE's fp32r
    path truncates operands to ~FP22, so pre-rounding on the host turns
    that truncation into a no-op and halves the effective input error."""
    u = a.view(np.uint32)
    lsb = (u >> np.uint32(10)) & np.uint32(1)
    return ((u + np.uint32(0x1FF) + lsb) & np.uint32(0xFFFFFC00)).view(np.float32)


def _in_maps(emb1, emb2, r):
    rt = _r22(np.ascontiguousarray(r.T))
    e1t = _r22(np.ascontiguousarray(emb1.T))
    e2t = _r22(np.ascontiguousarray(emb2.T))
    maps = []
    for k in range(8):
        a, b = k // G2, k % G2
        maps.append(
            {
                "e1": np.ascontiguousarray(e1t[:, a * M1 : (a + 1) * M1]),
                "e2": np.ascontiguousarray(e2t[:, b * M2 : (b + 1) * M2]),
                "rr": rt,
            }
        )
    return maps


def _install_profile_hook():
    """The agent image's antenv lacks axon_hooks; synthesize it so
    run_bass_kernel_spmd(trace=True) can reach the NTFF profiler."""
    import types

    if "antenv.axon_hooks" in sys.modules:
        return
    try:
        from trn_agent_boot.trn_boot import _ntff_profile_via_ctypes

        hook = _ntff_profile_via_ctypes("/opt/axon/libaxon_pjrt.so")
        mod = types.ModuleType("antenv.axon_hooks")
        mod.get_axon_ntff_profile_hook = lambda: hook
        sys.modules["antenv.axon_hooks"] = mod

        from concourse import bass_utils as _bu

        _orig_upload = _bu.upload_artifacts

        def _safe_upload(tmpdir):
            try:
                return _orig_upload(tmpdir)
            except Exception as e:  # no bucket access in this container
                return f"upload-skipped: {e}"

        _bu.upload_artifacts = _safe_upload
    except Exception:
        pass


def kernel(emb1, emb2, r, pi, _trace=False, _tmpdir=None):
    emb1 = np.asarray(emb1, dtype=np.float32)
    emb2 = np.asarray(emb2, dtype=np.float32)
    r = np.asarray(r, dtype=np.float32)
    # u codes +/-1, v codes +/-0.5: psum P = 0.5 * s_u.s_v, out = sin((pi/B)*P)
    scale = float(np.asarray(pi).reshape(-1)[0]) / B

    nc = _build(scale)
    if _trace:
        _install_profile_hook()
    try:
        res = run_bass_kernel_spmd(
            nc, _in_maps(emb1, emb2, r), list(range(8)), trace=_trace, tmpdir=_tmpdir
        )
    except ModuleNotFoundError:
        res = run_bass_kernel_spmd(nc, _in_maps(emb1, emb2, r), list(range(8)))

    full = np.empty((N1, N2), dtype=np.float32)
    for k in range(8):
        a, b = k // G2, k % G2
        full[a * M1 : (a + 1) * M1, b * M2 : (b + 1) * M2] = np.asarray(
            res.results[k]["out"]
        ).astype(np.float32)
    if _trace:
        kernel._last_exec_time_ns = res.exec_time_ns
    return full

